# revision 29
# baseline (speedup 1.0000x reference)
"""Trainium2 Bass kernel for nn_BaselineRvNNModel (collapsed RvNN/TreeLSTM).

Math (reference collapses to a per-node MLP + mean pool + classifier;
edge_index is dead):
    h1 = relu(x @ W1.T + b1)                      [N, H]
    g  = h1 @ W2.T + b2                           [N, H]   (pre-LN)
    gn = (g - mu) * rsqrt(var + eps)              per-row LN core
    iou = (gn * ln_w) @ W_iou.T + (ln_b @ W_iou.T + b_wiou + b_uiou)
    i, o, u = split(iou); c = sig(i)*tanh(u); hn = sig(o)*tanh(c)
    pooled = mean_rows(hn);  out = relu(pooled @ Wc1.T + bc1) @ Wc2.T + bc2

Distribution: data-parallel over nodes, 12500 rows/core on 8 cores,
AllReduce of the [H] pooled partial sum, replicated classifier.

Device layout: channels on partitions, rows on the free axis. x is
pre-transposed (and pre-tiled) host-side so no on-device transposes are
needed. LayerNorm's channel reduction is done with ones-vector matmuls on
the PE; rsqrt is computed as exp(-0.5*ln(v)) to stay within one extra ACT
table set; per-row stats are broadcast across partitions on GPSIMD.
"""

import numpy as np
import ml_dtypes

N_TOTAL = 100000
D = 768
H = 256
C = 4
NCORES = 8
LN_EPS = 1e-5

_CACHE = {}


def build_nc(npc, nt, ncores, use_f32_x=False, debug_taps=False, stage="full"):
    """Build the per-core Bass graph. npc = rows per core, nt = rows per tile."""
    from contextlib import ExitStack
    import concourse.bass as bass
    import concourse.bacc as bacc
    import concourse.tile as tile
    from concourse import mybir

    f32 = mybir.dt.float32
    f32r = mybir.dt.float32r
    bf16 = mybir.dt.bfloat16
    AF = mybir.ActivationFunctionType
    ALU = mybir.AluOpType

    ntiles = npc // nt
    assert ntiles * nt == npc
    KD = D // 128   # 6 contraction chunks for x
    KH = H // 128   # 2 chunks for H
    K3 = 3 * H // 128  # 6 output chunks for iou

    xdt = f32r if use_f32_x else bf16

    nc = bacc.Bacc("TRN2", target_bir_lowering=False, debug=False,
                   num_devices=ncores)

    # DRAM inputs. xtt is pre-tiled host-side: [ntiles, 128, KD, nt]
    xtt = nc.dram_tensor("xtt", [ntiles, 128, KD, nt], xdt, kind="ExternalInput")
    w1t = nc.dram_tensor("w1t", [D, H], xdt, kind="ExternalInput")        # W1.T
    b1d = nc.dram_tensor("b1d", [128, KH], f32, kind="ExternalInput")
    w2t = nc.dram_tensor("w2t", [H, H], bf16, kind="ExternalInput")       # W2.T
    b2d = nc.dram_tensor("b2d", [128, KH], f32, kind="ExternalInput")
    wiout = nc.dram_tensor("wiout", [H, 3 * H], bf16, kind="ExternalInput")  # (W_iou*ln_w).T
    c3d = nc.dram_tensor("c3d", [128, K3], f32, kind="ExternalInput")
    wc1t = nc.dram_tensor("wc1t", [H, H // 2], f32, kind="ExternalInput")  # Wc1.T/N
    bc1d = nc.dram_tensor("bc1d", [128, 1], f32, kind="ExternalInput")
    wc2t = nc.dram_tensor("wc2t", [H // 2, C], f32, kind="ExternalInput")  # Wc2.T
    bc2d = nc.dram_tensor("bc2d", [C, 1], f32, kind="ExternalInput")
    out_d = nc.dram_tensor("out", [C, 1], f32, kind="ExternalOutput")
    if debug_taps:
        dbg_g = nc.dram_tensor("dbg_g", [128, H // 128, npc], bf16,
                               kind="ExternalOutput")
        dbg_sst = nc.dram_tensor("dbg_sst", [npc // nt, 2, nt], bf16,
                                 kind="ExternalOutput")
        dbg_pool = nc.dram_tensor("dbg_pool", [128, H // 128, npc // nt], f32,
                                  kind="ExternalOutput")

    with tile.TileContext(nc) as tc, ExitStack() as ctx:
        # ---------------- constants (live whole kernel) ----------------
        pconst = ctx.enter_context(tc.tile_pool(name="consts", bufs=1))
        w1_sb = pconst.tile([128, KD, H], xdt)          # [128, k, m-chans]
        nc.sync.dma_start(w1_sb[:], w1t.ap().rearrange("(k p) m -> p k m", p=128))
        w2_sb = pconst.tile([128, KH, H], bf16)
        nc.sync.dma_start(w2_sb[:], w2t.ap().rearrange("(k p) m -> p k m", p=128))
        w3_sb = pconst.tile([128, KH, 3 * H], bf16)
        nc.sync.dma_start(w3_sb[:], wiout.ap().rearrange("(k p) m -> p k m", p=128))
        b1_sb = pconst.tile([128, KH], f32)
        nc.sync.dma_start(b1_sb[:], b1d.ap())
        b2_sb = pconst.tile([128, KH], f32)
        nc.sync.dma_start(b2_sb[:], b2d.ap())
        c3_sb = pconst.tile([128, K3], f32)
        nc.sync.dma_start(c3_sb[:], c3d.ap())
        wc1_sb = pconst.tile([128, KH, H // 2], f32)
        nc.sync.dma_start(wc1_sb[:], wc1t.ap().rearrange("(k p) m -> p k m", p=128))
        bc1_sb = pconst.tile([128, 1], f32)
        nc.sync.dma_start(bc1_sb[:], bc1d.ap())
        wc2_sb = pconst.tile([128, C], f32)
        nc.sync.dma_start(wc2_sb[:], wc2t.ap())
        bc2_sb = pconst.tile([C, 1], f32)
        nc.sync.dma_start(bc2_sb[:], bc2d.ap())
        ones_sb = pconst.tile([128, 1], bf16)
        nc.vector.memset(ones_sb[:], 1.0 / H)
        eps_sb = pconst.tile([1, 1], f32)
        nc.vector.memset(eps_sb[:], LN_EPS)

        # persistent buffers
        pg = ctx.enter_context(tc.tile_pool(name="gbuf", bufs=1))
        gbuf = pg.tile([128, KH, npc], bf16)            # pre-LN activations
        accb = pg.tile([128, KH, nt], f32)              # pooled row accumulators
        nc.vector.memset(accb[:], 0.0)

        pdram = ctx.enter_context(tc.tile_pool(name="dram", bufs=1, space="DRAM"))
        statsd = pdram.tile([2, ntiles * nt], f32)
        ssd = pdram.tile([ntiles, 2, nt], bf16)         # s / s*mu rows
        ccin = pdram.tile([128, KH], f32)
        ccout = pdram.tile([128, KH], f32)

        # ================= phase A: mm1, mm2, row stats =================
        with tc.tile_pool(name="xin", bufs=3) as px, \
             tc.tile_pool(name="h1", bufs=4) as ph1, \
             tc.tile_pool(name="gsq", bufs=4) as pgs, \
             tc.tile_pool(name="stage", bufs=4) as pstg, \
             tc.tile_pool(name="psA1", bufs=3, space="PSUM") as pps1, \
             tc.tile_pool(name="psA2", bufs=2, space="PSUM") as pps2, \
             tc.tile_pool(name="psAs", bufs=3, space="PSUM") as ppss:
            for j in range(ntiles):
                xs = px.tile([128, KD, nt], xdt, tag="x")
                nc.sync.dma_start(xs[:], xtt.ap()[j])
                h1s = []
                for m in range(KH):
                    pm = pps1.tile([128, nt], f32, tag="h1p")
                    for k in range(KD):
                        nc.tensor.matmul(
                            pm[:],
                            w1_sb[:, k, m * 128:(m + 1) * 128],
                            xs[:, k, :],
                            start=(k == 0), stop=(k == KD - 1))
                    h1 = ph1.tile([128, nt], bf16, tag="h1")
                    # relu(h + b1) on DVE: (psum + b1) max 0
                    nc.vector.tensor_scalar(
                        out=h1[:], in0=pm[:], scalar1=b1_sb[:, m:m + 1],
                        scalar2=0.0, op0=ALU.add, op1=ALU.max)
                    h1s.append(h1)
                for m in range(KH):
                    pm = pps2.tile([128, nt], f32, tag="h2p")
                    for k in range(KH):
                        nc.tensor.matmul(
                            pm[:], w2_sb[:, k, m * 128:(m + 1) * 128],
                            h1s[k][:], start=(k == 0), stop=(k == KH - 1))
                    gv = gbuf[:, m, j * nt:(j + 1) * nt]
                    nc.vector.tensor_scalar(
                        out=gv, in0=pm[:], scalar1=b2_sb[:, m:m + 1],
                        scalar2=None, op0=ALU.add)
                gsq = pgs.tile([128, KH, nt], bf16, tag="gsq")
                for m in range(KH):
                    nc.vector.tensor_tensor(
                        out=gsq[:, m, :], in0=gbuf[:, m, j * nt:(j + 1) * nt],
                        in1=gbuf[:, m, j * nt:(j + 1) * nt], op=ALU.mult)
                pmu = ppss.tile([1, nt], f32, tag="stat")
                for m in range(KH):
                    nc.tensor.matmul(pmu[:], ones_sb[:],
                                     gbuf[:, m, j * nt:(j + 1) * nt],
                                     start=(m == 0), stop=(m == KH - 1))
                pmsq = ppss.tile([1, nt], f32, tag="stat")
                for m in range(KH):
                    nc.tensor.matmul(pmsq[:], ones_sb[:], gsq[:, m, :],
                                     start=(m == 0), stop=(m == KH - 1))
                smu = pstg.tile([1, nt], f32, tag="smu")
                nc.scalar.activation(smu[:], pmu[:], AF.Copy)
                nc.sync.dma_start(statsd[0:1, j * nt:(j + 1) * nt], smu[:])
                sms = pstg.tile([1, nt], f32, tag="sms")
                # msq + eps (eps folded here so var' = msq' - mu^2 = var + eps)
                nc.scalar.activation(sms[:], pmsq[:], AF.Identity,
                                     bias=eps_sb[:])
                nc.sync.dma_start(statsd[1:2, j * nt:(j + 1) * nt], sms[:])

        if stage == "A":
            nc.sync.dma_start(out_d.ap(), statsd[0:1, 0:C])

        # ============ phase boundary: s = exp(-0.5*ln(var+eps)) ============
        if stage != "A":
          with tc.tile_pool(name="stats", bufs=1) as pst:
            mu2 = pst.tile([ntiles, nt], f32)
            nc.sync.dma_start(
                mu2[:], statsd[0:1, :].rearrange("o (j t) -> (o j) t", j=ntiles))
            msq2 = pst.tile([ntiles, nt], f32)
            nc.sync.dma_start(
                msq2[:], statsd[1:2, :].rearrange("o (j t) -> (o j) t", j=ntiles))
            musq = pst.tile([ntiles, nt], f32)
            nc.scalar.activation(musq[:], mu2[:], AF.Square)
            varr = pst.tile([ntiles, nt], f32)
            nc.vector.tensor_tensor(out=varr[:], in0=msq2[:], in1=musq[:],
                                    op=ALU.subtract)
            lnv = pst.tile([ntiles, nt], f32)
            nc.scalar.activation(lnv[:], varr[:], AF.Ln)
            sst = pst.tile([ntiles, 2, nt], bf16)
            nc.scalar.activation(sst[:, 0, :], lnv[:], AF.Exp, scale=-0.5)
            nc.vector.tensor_tensor(out=sst[:, 1, :], in0=sst[:, 0, :],
                                    in1=mu2[:], op=ALU.mult)
            nc.sync.dma_start(ssd[:], sst[:])

        # ================= phase B: LN apply, mm3, gates =================
        if stage != "A":
          with tc.tile_pool(name="gn", bufs=4) as pgn, \
             tc.tile_pool(name="gt", bufs=8) as pgt, \
             tc.tile_pool(name="hnscr", bufs=2) as phs, \
             tc.tile_pool(name="psB", bufs=6, space="PSUM") as ppsb:
            for j in range(ntiles):
                jw = slice(j * nt, (j + 1) * nt)
                sb = pgn.tile([128, 2, nt], bf16, tag="sb")
                for t in range(2):
                    nc.sync.dma_start(
                        sb[:, t, :], ssd[j:j + 1, t, :].partition_broadcast(128))
                gn = pgn.tile([128, KH, nt], bf16, tag="gn")
                for m in range(KH):
                    tt = pgt.tile([128, nt], bf16, tag="tmp")
                    nc.vector.tensor_tensor(out=tt[:], in0=gbuf[:, m, jw],
                                            in1=sb[:, 0, :], op=ALU.mult)
                    nc.vector.tensor_tensor(out=gn[:, m, :], in0=tt[:],
                                            in1=sb[:, 1, :],
                                            op=ALU.subtract)
                for m in range(KH):
                    pious = []
                    for m3 in (m, 2 + m, 4 + m):
                        pio = ppsb.tile([128, nt], f32, tag="iou")
                        for k in range(KH):
                            nc.tensor.matmul(
                                pio[:], w3_sb[:, k, m3 * 128:(m3 + 1) * 128],
                                gn[:, k, :], start=(k == 0), stop=(k == KH - 1))
                        pious.append(pio)
                    pi, po, pu = pious
                    si = pgt.tile([128, nt], bf16, tag="si")
                    nc.scalar.activation(si[:], pi[:], AF.Sigmoid,
                                         bias=c3_sb[:, m:m + 1])
                    tu = pgt.tile([128, nt], bf16, tag="tu")
                    nc.scalar.activation(tu[:], pu[:], AF.Tanh,
                                         bias=c3_sb[:, 4 + m:5 + m])
                    so = pgt.tile([128, nt], bf16, tag="so")
                    nc.scalar.activation(so[:], po[:], AF.Sigmoid,
                                         bias=c3_sb[:, 2 + m:3 + m])
                    cpre = pgt.tile([128, nt], bf16, tag="cpre")
                    nc.vector.tensor_tensor(out=cpre[:], in0=si[:], in1=tu[:],
                                            op=ALU.mult)
                    tc_t = pgt.tile([128, nt], bf16, tag="tc")
                    nc.scalar.activation(tc_t[:], cpre[:], AF.Tanh)
                    hns = phs.tile([128, nt], bf16, tag="hns")
                    nc.vector.tensor_tensor(out=hns[:], in0=so[:], in1=tc_t[:],
                                            op=ALU.mult)
                    nc.vector.tensor_tensor(out=accb[:, m, :],
                                            in0=accb[:, m, :], in1=hns[:],
                                            op=ALU.add)

        # ================= pool + all-reduce + classifier =================
        if debug_taps:
            nc.sync.dma_start(dbg_g.ap(), gbuf[:])
            nc.sync.dma_start(dbg_sst.ap(), ssd[:])
            nc.sync.dma_start(dbg_pool.ap(), accb[:, :, 0:ntiles])

        if stage == "B":
            nc.sync.dma_start(out_d.ap(), accb[0:C, 0, 0:1])

        if stage in ("full", "noar"):
          with tc.tile_pool(name="fin", bufs=1) as pf, \
             tc.tile_pool(name="psF", bufs=2, space="PSUM") as ppsf:
            pv = pf.tile([128, KH], f32)
            for m in range(KH):
                nc.vector.tensor_reduce(out=pv[:, m:m + 1], in_=accb[:, m, :],
                                        axis=mybir.AxisListType.X,
                                        op=ALU.add)
            nc.sync.dma_start(ccin[:], pv[:])
            if stage == "noar":
                nc.sync.dma_start(ccout[:], ccin[:])
            else:
                nc.gpsimd.collective_compute(
                    "AllReduce", ALU.add,
                    replica_groups=[list(range(ncores))],
                    ins=[ccin[:].opt()], outs=[ccout[:].opt()])
            ps = pf.tile([128, KH], f32)
            nc.sync.dma_start(ps[:], ccout[:])
            pz = ppsf.tile([128, 1], f32)
            for k in range(KH):
                nc.tensor.matmul(pz[:], wc1_sb[:, k, :], ps[:, k:k + 1],
                                 start=(k == 0), stop=(k == KH - 1))
            zz = pf.tile([128, 1], f32)
            nc.vector.tensor_scalar(out=zz[:], in0=pz[:], scalar1=bc1_sb[:],
                                    scalar2=0.0, op0=ALU.add, op1=ALU.max)
            po2 = ppsf.tile([C, 1], f32)
            nc.tensor.matmul(po2[:], wc2_sb[:], zz[:], start=True, stop=True)
            oo = pf.tile([C, 1], f32)
            nc.vector.tensor_scalar(out=oo[:], in0=po2[:], scalar1=bc2_sb[:],
                                    scalar2=None, op0=ALU.add)
            nc.sync.dma_start(out_d.ap(), oo[:])

    nc.compile()
    return nc


def host_prep(inputs, npc, nt, ncores, use_f32_x=False):
    """Shard + lay out inputs for the device. Returns in_maps (list per core)."""
    bf16 = ml_dtypes.bfloat16
    xdt = np.float32 if use_f32_x else bf16
    ntiles = npc // nt
    KH = H // 128
    K3 = 3 * H // 128

    x = np.asarray(inputs["x"], np.float32)
    W1 = np.asarray(inputs["W1"], np.float32)
    b1 = np.asarray(inputs["b1"], np.float32)
    W2 = np.asarray(inputs["W2"], np.float32)
    b2 = np.asarray(inputs["b2"], np.float32)
    ln_w = np.asarray(inputs["ln_w"], np.float32)
    ln_b = np.asarray(inputs["ln_b"], np.float32)
    W_iou = np.asarray(inputs["W_iou"], np.float32)
    b_wiou = np.asarray(inputs["b_wiou"], np.float32)
    b_uiou = np.asarray(inputs["b_uiou"], np.float32)
    Wc1 = np.asarray(inputs["Wc1"], np.float32)
    bc1 = np.asarray(inputs["bc1"], np.float32)
    Wc2 = np.asarray(inputs["Wc2"], np.float32)
    bc2 = np.asarray(inputs["bc2"], np.float32)

    shared = {
        "w1t": np.ascontiguousarray(W1.T).astype(xdt),
        "b1d": np.ascontiguousarray(b1.reshape(KH, 128).T),
        "w2t": np.ascontiguousarray(W2.T).astype(bf16),
        "b2d": np.ascontiguousarray(b2.reshape(KH, 128).T),
        "wiout": np.ascontiguousarray((W_iou * ln_w[None, :]).T).astype(bf16),
        "c3d": np.ascontiguousarray(
            (W_iou @ ln_b + b_wiou + b_uiou).astype(np.float32)
            .reshape(K3, 128).T),
        "wc1t": np.ascontiguousarray(Wc1.T).astype(np.float32) / float(x.shape[0]),
        "bc1d": np.ascontiguousarray(bc1.reshape(128, 1)),
        "wc2t": np.ascontiguousarray(Wc2.T).astype(np.float32),
        "bc2d": np.ascontiguousarray(bc2.reshape(C, 1)),
    }
    in_maps = []
    for c in range(ncores):
        xs = x[c * npc:(c + 1) * npc]                      # [npc, D]
        # [ntiles, 128, KD, nt]: tile j, partition p, d-chunk k, row t
        xtt = (xs.reshape(ntiles, nt, D // 128, 128)
               .transpose(0, 3, 2, 1).astype(xdt))
        in_maps.append({"xtt": np.ascontiguousarray(xtt), **shared})
    return in_maps


def kernel(**inputs):
    from concourse.bass_utils import run_bass_kernel_spmd

    npc = N_TOTAL // NCORES
    nt = 500
    key = (npc, nt, NCORES)
    if key not in _CACHE:
        _CACHE[key] = build_nc(npc, nt, NCORES)
    nc = _CACHE[key]
    in_maps = host_prep(inputs, npc, nt, NCORES)
    res = run_bass_kernel_spmd(nc, in_maps, core_ids=list(range(NCORES)))
    return np.ascontiguousarray(
        res.results[0]["out"].reshape(1, C).astype(np.float32))
